# revision 1
# baseline (speedup 1.0000x reference)
"""Cross-attention kernel for Trainium2, 8 NeuronCores, data-parallel over
batch (B=8 == n_cores, one batch element per core, no collectives).

Design (evolved from a 584us f32r baseline to ~266us measured):
  - x^T / enc^T transposed and bf16-converted on HOST; all matmul operands
    are bf16 (true 1 cyc/row on the PE at 2.4GHz -- f32r runs in slow
    fp32_mode=HIGH on this hardware).  PSUM accumulation stays f32.
  - S^T = K_h @ Q_h^T per 128-i-chunk into [128,1024] psum tiles (both
    t-halves) -> ONE exp per chunk on the Act engine; the 64-row i-tail of
    each head PAIR shares a single psum tile + exp (V tail rows duplicated
    to partitions 64:128 via SBUF->SBUF DMA so matmul base partitions match).
  - Softmax denominator via ones-augmented V (row 64 of the AV psum = Z).
    1/Z staged per head pair: Z rows at partitions 0/32 of one tile, ONE
    reciprocal_approx_fast + ONE bf16 round per pair (their cost depends
    only on free size), rank-1 PE broadcast, in-place YT scale on DVE.
  - Emission pipelines each pair's S/exp a FULL pair ahead of its AV and
    interleaves the S psum allocations 1:1 with other matmuls so the
    rotating 4x[128,1024] PSUM pool never blocks on the Act engine.
  - bv bias folded into the V-proj drain (broadcast tile, tensor_add);
    bo applied as a rank-1 PE accumulate; per-partition bq/bk fused into
    the Q/K psum drains (tensor_scalar_add).
  - Input DMAs split across SP/Act/GpSimd DGE rings by first use; wo reuses
    wv's SBUF slots; output written as bf16 (halves the out DMA).
  - PE p-state warmup matmuls run while the first input DMAs stream.
"""

import numpy as np

T = 1024
C = 1024
I = 576
H = 16
D = 64
NCC = C // 128          # 8 contraction chunks
NIC = (I + 127) // 128  # 5 i chunks (128,128,128,128,64)
I_CH = [128, 128, 128, 128, 64]
VW = 66                 # per-head column block in V tile: 64 v cols + ones col + pad
SCALE = 1.0 / np.sqrt(D)

_CACHE = {}


def _build_nc():
    import concourse.bass as bass
    import concourse.bacc as bacc
    import concourse.mybir as mybir
    import concourse.tile as tile
    from contextlib import ExitStack

    f32 = mybir.dt.float32
    f32r = mybir.dt.float32r
    bf16 = mybir.dt.bfloat16

    nc = bacc.Bacc()

    xT_d = nc.dram_tensor("xT", [C, T], bf16, kind="ExternalInput")
    encT_d = nc.dram_tensor("encT", [C, I], bf16, kind="ExternalInput")
    wqT_d = nc.dram_tensor("wqT", [C, C], bf16, kind="ExternalInput")
    wkT_d = nc.dram_tensor("wkT", [C, C], bf16, kind="ExternalInput")
    wvT_d = nc.dram_tensor("wvT", [C, C], bf16, kind="ExternalInput")
    woT_d = nc.dram_tensor("woT", [C, C], bf16, kind="ExternalInput")
    bq_d = nc.dram_tensor("bq", [C], f32, kind="ExternalInput")
    bk_d = nc.dram_tensor("bk", [C], f32, kind="ExternalInput")
    bvb_d = nc.dram_tensor("bvb", [128, C], f32, kind="ExternalInput")
    bob_d = nc.dram_tensor("bob", [1, C], bf16, kind="ExternalInput")
    out_d = nc.dram_tensor("out", [T, C], bf16, kind="ExternalOutput")

    with ExitStack() as ctx:
        tc = ctx.enter_context(tile.TileContext(nc))

        resid = ctx.enter_context(tc.tile_pool(name="resid", bufs=1))
        misc = ctx.enter_context(tc.tile_pool(name="misc", bufs=1))
        ps = ctx.enter_context(tc.tile_pool(name="ps", bufs=4, space="PSUM"))
        exps = ctx.enter_context(tc.tile_pool(name="exps", bufs=22))
        osb = ctx.enter_context(tc.tile_pool(name="osb", bufs=2))
        wvp = ctx.enter_context(tc.tile_pool(name="wvp", bufs=1))

        # constants
        ones33 = misc.tile([33, 64], bf16)
        nc.gpsimd.memset(ones33, 1.0)
        bq_t = misc.tile([128, NCC], f32)
        nc.sync.dma_start(out=bq_t, in_=bq_d[:].rearrange("(oc p) -> p oc", p=128))
        bk_t = misc.tile([128, NCC], f32)
        nc.sync.dma_start(out=bk_t, in_=bk_d[:].rearrange("(oc p) -> p oc", p=128))
        bvb = misc.tile([128, C], f32)
        nc.sync.dma_start(out=bvb, in_=bvb_d[:, :])
        bobr = misc.tile([1, C], bf16)
        nc.sync.dma_start(out=bobr, in_=bob_d[:, :])
        ones128 = misc.tile([1, 128], bf16)
        nc.gpsimd.memset(ones128, 1.0)
        # per-head 1/Z rows ([1, T] f32 at partition 0; recip runs in place,
        # rank-1 broadcast reads them bitcast as f32r).  Only ~4 heads are
        # ever in flight between staging and apply, so rotate 6 buffers.
        zrp = ctx.enter_context(tc.tile_pool(name="zrp", bufs=3))
        zpair = {}

        # resident tensors
        xT = [resid.tile([128, T], bf16, tag=f"xT{i}", name=f"xT{i}") for i in range(NCC)]
        encT = [resid.tile([128, I], bf16, tag=f"encT{i}", name=f"encT{i}") for i in range(NCC)]
        QT = [resid.tile([128, T], bf16, tag=f"QT{i}", name=f"QT{i}") for i in range(NCC)]
        KT = [resid.tile([128, I], bf16, tag=f"KT{i}", name=f"KT{i}") for i in range(NCC)]
        V3 = [resid.tile([128, H, VW], bf16, tag=f"V{i}", name=f"V{i}") for i in range(NIC)]
        YT = [resid.tile([128, T], bf16, tag=f"YT{i}", name=f"YT{i}") for i in range(NCC)]
        wq = [resid.tile([128, C], bf16, tag=f"wq{i}", name=f"wq{i}") for i in range(NCC)]
        wk = [resid.tile([128, C], bf16, tag=f"wk{i}", name=f"wk{i}") for i in range(NCC)]
        # wo reuses wv's SBUF slots (same tag, bufs=1): its DMA waits until
        # the last V-proj matmul has consumed wv[cc].
        wv = [wvp.tile([128, C], bf16, tag=f"wv{i}", name=f"wv{i}") for i in range(NCC)]

        # input DMAs, ordered by first use (K proj first, V proj deferred),
        # split across SP + Activation DGE rings; wk rides the GpSimd SWDGE
        # ring as a third channel (slow ~1us/issue, but the Pool engine is
        # idle and the transfers overlap the other rings)
        def ld(cc, dst, dram):
            eng = nc.sync if cc % 2 == 0 else nc.scalar
            eng.dma_start(out=dst[cc], in_=dram[cc * 128 : (cc + 1) * 128])
        for cc in range(NCC):
            nc.gpsimd.dma_start(out=wk[cc], in_=wkT_d[cc * 128 : (cc + 1) * 128])
        for cc in range(NCC):
            ld(cc, encT, encT_d)
        for cc in range(NCC):
            ld(cc, xT, xT_d)
            ld(cc, wq, wqT_d)
            ld(cc, wv, wvT_d)
        # ones columns of V3 (col 64 of each head block)
        for ii in range(NIC):
            nc.gpsimd.memset(V3[ii][:, :, 64:65], 1.0)

        # PE p-state warmup: the PE ramps to full clock only after ~3us of
        # continuous work; burn dummy matmuls while the input DMAs stream so
        # the real pipeline starts at 2.4GHz.
        warm = misc.tile([1, 512], bf16)
        nc.gpsimd.memset(warm, 1.0)
        for w in range(20):
            pw = ps.tile([128, 1024], f32, tag="ps")
            nc.tensor.matmul(pw[:64, 0:512], ones33[0:1, :], warm,
                             start=True, stop=True)

        def v_proj(och):
            """V3[ii][:, h, 0:64] = (enc @ WvT + bv), heads 8*och..8*och+7."""
            osl = slice(och * 512, (och + 1) * 512)
            for ii in range(NIC):
                pi = I_CH[ii]
                pv = ps.tile([128, 1024], f32, tag="ps")
                for cc in range(NCC):
                    nc.tensor.matmul(
                        pv[:pi, 0:512],
                        encT[cc][:, ii * 128 : ii * 128 + pi],
                        wv[cc][:, osl],
                        start=(cc == 0),
                        stop=(cc == NCC - 1),
                    )
                nc.vector.tensor_add(
                    V3[ii][:pi, och * 8 : och * 8 + 8, 0:64],
                    pv[:pi, 0:512].rearrange("p (h d) -> p h d", d=64),
                    bvb[:pi, osl].rearrange("p (h d) -> p h d", d=64),
                )
            # duplicate the i-tail rows to partitions 64:128 so the odd head
            # of each pair reads them at base partition 64 (DMA: engines are
            # lane-locked and cannot shift partitions)
            nc.sync.dma_start(
                out=V3[4][64:128, och * 8 : och * 8 + 8, :],
                in_=V3[4][0:64, och * 8 : och * 8 + 8, :],
            )

        def k_proj(oc):
            pk = ps.tile([128, 1024], f32, tag="ps")
            for cc in range(NCC):
                for ih in range(2):
                    nc.tensor.matmul(
                        pk[:, ih * 512 : ih * 512 + 288],
                        wk[cc][:, oc * 128 : (oc + 1) * 128],
                        encT[cc][:, ih * 288 : (ih + 1) * 288],
                        start=(cc == 0),
                        stop=(cc == NCC - 1),
                    )
            for ih in range(2):
                nc.vector.tensor_scalar_add(
                    KT[oc][:, ih * 288 : (ih + 1) * 288],
                    pk[:, ih * 512 : ih * 512 + 288],
                    bk_t[:, oc : oc + 1],
                )

        def q_proj_t(oc, tch):
            if True:
                pq = ps.tile([128, 1024], f32, tag="ps")
                for cc in range(NCC):
                    nc.tensor.matmul(
                        pq[:, 0:512],
                        wq[cc][:, oc * 128 : (oc + 1) * 128],
                        xT[cc][:, tch * 512 : (tch + 1) * 512],
                        start=(cc == 0),
                        stop=(cc == NCC - 1),
                    )
                nc.vector.tensor_scalar_add(
                    QT[oc][:, tch * 512 : (tch + 1) * 512],
                    pq[:, 0:512],
                    bq_t[:, oc : oc + 1],
                )

        def s_chunk(h, ii):
            """S^T = K_h @ Q_h^T for one full i-chunk (both t-halves) + exp."""
            oc, hb = h // 2, (h % 2) * 64
            sp = ps.tile([128, 1024], f32, tag="ps")
            for tch in range(2):
                nc.tensor.matmul(
                    sp[:, tch * 512 : (tch + 1) * 512],
                    KT[oc][hb : hb + 64, ii * 128 : (ii + 1) * 128],
                    QT[oc][hb : hb + 64, tch * 512 : (tch + 1) * 512],
                    start=True,
                    stop=True,
                )
            e = exps.tile([128, 1024], bf16, tag="exps")
            nc.scalar.activation(
                e, sp, mybir.ActivationFunctionType.Exp, scale=float(SCALE)
            )
            return e

        def s_tail_pair(oc):
            """The 64-row i-tail (ii=4) of BOTH heads of chunk oc shares one
            psum tile (head 2oc at partitions 0:64, head 2oc+1 at 64:128) and
            one exp instruction.  The odd head reads the duplicated V3 tail
            rows 64:128 so matmul base partitions match."""
            sp = ps.tile([128, 1024], f32, tag="ps")
            for hb in (0, 64):
                for tch in range(2):
                    nc.tensor.matmul(
                        sp[hb : hb + 64, tch * 512 : (tch + 1) * 512],
                        KT[oc][hb : hb + 64, 512:576],
                        QT[oc][hb : hb + 64, tch * 512 : (tch + 1) * 512],
                        start=True,
                        stop=True,
                    )
            e = exps.tile([128, 1024], bf16, tag="exps")
            nc.scalar.activation(
                e, sp, mybir.ActivationFunctionType.Exp, scale=float(SCALE)
            )
            return e

        def av(h, es, etail, y_on_act=False):
            """y^T rows + Z row via ones-augmented V; unnormalized drain."""
            oc, hb = h // 2, (h % 2) * 64
            py = ps.tile([128, 1024], f32, tag="ps")
            for tch in range(2):
                tsl = slice(tch * 512, (tch + 1) * 512)
                for ii in range(4):
                    nc.tensor.matmul(
                        py[:65, tsl],
                        V3[ii][:, h, 0:65],
                        es[ii][:, tsl],
                        start=(ii == 0),
                        stop=False,
                    )
                nc.tensor.matmul(
                    py[:65, tsl],
                    V3[4][hb : hb + 64, h, 0:65],
                    etail[hb : hb + 64, tsl],
                    start=False,
                    stop=True,
                )
            # stage Z on DVE: even head -> partition 0, odd head -> partition
            # 32 of the pair tile (engine writes must start at 0/32/64/96).
            # After the odd head: ONE approx-recip + ONE bf16 round per pair
            # (their cost depends only on free size, not partition count).
            if h % 2 == 0:
                zpair[oc] = [zrp.tile([33, T], f32, tag="ztmp", name=f"zt2_{oc}"), None]
                nc.vector.tensor_copy(zpair[oc][0][0:1, :], py[64:65, :])
            else:
                zt2 = zpair[oc][0]
                nc.vector.tensor_copy(zt2[32:33, :], py[64:65, :])
                nc.vector.reciprocal_approx_fast(out=zt2, in_=zt2)
                zr2 = zrp.tile([33, T], bf16, tag="zr", name=f"zr{oc}")
                nc.vector.tensor_copy(zr2, zt2)
                zpair[oc][1] = zr2
            if y_on_act:
                # kernel tail: Act is idle once the last exp retires; keep the
                # DVE free for the 1/Z chain that gates the final applies
                nc.scalar.copy(YT[oc][hb : hb + 64, :], py[:64, :])
            else:
                nc.vector.tensor_copy(YT[oc][hb : hb + 64, :], py[:64, :])

        def apply_norm(h):
            """rank-1 broadcast of staged 1/Z; scale YT in place."""
            oc, hb = h // 2, (h % 2) * 64
            zb = (h % 2) * 32   # even head's 1/Z at partition 0, odd at 32
            zr2 = zpair[oc][1]
            pb = ps.tile([128, 1024], f32, tag="ps")
            for tch in range(2):
                tsl = slice(tch * 512, (tch + 1) * 512)
                nc.tensor.matmul(
                    pb[:64, tsl],
                    ones33[zb : zb + 1, :],
                    zr2[zb : zb + 1, tsl],
                    start=True, stop=True,
                )
            nc.vector.tensor_mul(
                YT[oc][hb : hb + 64, :], YT[oc][hb : hb + 64, :], pb[:64, :]
            )

        # ---- projection + attention pipeline ----
        # Attention starts BEFORE the V projection, each pair's S/exp is
        # emitted a FULL pair ahead of its AV, and the S psum allocations are
        # interleaved 1:1 with other matmuls so the rotating 4-tile PSUM pool
        # never blocks on the Act engine draining an S tile (the dominant
        # stall in earlier revisions).  apply_norm for pair oc runs during
        # oc+1 (its DVE staging chain long drained).
        P = {h: [] for h in range(H)}
        PT = {}
        k_proj(0)
        q_proj_t(0, 0)
        q_proj_t(0, 1)
        P[0] = [s_chunk(0, ii) for ii in range(4)]
        P[1] = [s_chunk(1, ii) for ii in range(4)]
        PT[0] = s_tail_pair(0)
        k_proj(1)
        q_proj_t(1, 0)
        q_proj_t(1, 1)
        P[2] = [s_chunk(2, ii) for ii in range(4)]
        P[3] = [s_chunk(3, ii) for ii in range(4)]
        PT[1] = s_tail_pair(1)
        v_proj(0)
        for oc in range(NCC):
            nxt = oc + 2
            h0n, h1n = 2 * nxt, 2 * nxt + 1
            live = nxt < NCC
            if live:
                k_proj(nxt)
                q_proj_t(nxt, 0)
                q_proj_t(nxt, 1)
                P[h0n].append(s_chunk(h0n, 0))
            if oc >= 1:
                apply_norm(2 * oc - 2)
            if live:
                P[h0n].append(s_chunk(h0n, 1))
            av(2 * oc, P[2 * oc], PT[oc], y_on_act=(oc == NCC - 1))
            if live:
                P[h0n].append(s_chunk(h0n, 2))
                P[h0n].append(s_chunk(h0n, 3))
            if oc >= 1:
                apply_norm(2 * oc - 1)
            if live:
                P[h1n].append(s_chunk(h1n, 0))
            av(2 * oc + 1, P[2 * oc + 1], PT[oc], y_on_act=(oc == NCC - 1))
            if live:
                P[h1n].append(s_chunk(h1n, 1))
                P[h1n].append(s_chunk(h1n, 2))
                P[h1n].append(s_chunk(h1n, 3))
                PT[nxt] = s_tail_pair(nxt)
            if oc == 0:
                v_proj(1)
        apply_norm(14)
        apply_norm(15)

        # wo loads into wv's recycled slots (DMA is idle during attention)
        wo = [wvp.tile([128, C], bf16, tag=f"wv{i}", name=f"wo{i}") for i in range(NCC)]
        for cc in range(NCC):
            nc.sync.dma_start(out=wo[cc], in_=woT_d[cc * 128 : (cc + 1) * 128])

        # ---- output projection ----
        for tt in range(8):
            po = ps.tile([128, 1024], f32, tag="ps")
            for och in range(2):
                osl = slice(och * 512, (och + 1) * 512)
                for cc in range(NCC):
                    nc.tensor.matmul(
                        po[:, osl],
                        YT[cc][:, tt * 128 : (tt + 1) * 128],
                        wo[cc][:, osl],
                        start=(cc == 0),
                        stop=False,
                    )
                nc.tensor.matmul(
                    po[:, osl], ones128[0:1, :], bobr[0:1, osl],
                    start=False, stop=True,
                )
            ot = osb.tile([128, C], bf16, tag="osb")
            if tt % 2 == 0:
                nc.vector.tensor_copy(ot, po)
                nc.sync.dma_start(out=out_d[tt * 128 : (tt + 1) * 128], in_=ot)
            else:
                nc.scalar.copy(ot, po)
                nc.scalar.dma_start(out=out_d[tt * 128 : (tt + 1) * 128], in_=ot)

    nc.compile()
    return nc


def _get_nc():
    if "nc" not in _CACHE:
        _CACHE["nc"] = _build_nc()
    return _CACHE["nc"]


def _prep_in_maps(x, encoder_output, Wq, bq, Wkv, bkv, Wo, bo):
    import ml_dtypes
    f = np.float32
    bf = ml_dtypes.bfloat16
    x = np.asarray(x, f)
    enc = np.asarray(encoder_output, f)
    wqT = np.ascontiguousarray(np.asarray(Wq, f).T.astype(bf))
    wkv = np.asarray(Wkv, f)
    wkT = np.ascontiguousarray(wkv[:C].T.astype(bf))
    wvT = np.ascontiguousarray(wkv[C:].T.astype(bf))
    woT = np.ascontiguousarray(np.asarray(Wo, f).T.astype(bf))
    bq = np.asarray(bq, f)
    bkv = np.asarray(bkv, f)
    bo = np.asarray(bo, f)
    bvb = np.ascontiguousarray(np.broadcast_to(bkv[C:], (128, C)).astype(f))
    bob = np.ascontiguousarray(bo.reshape(1, C).astype(bf))
    shared = {
        "wqT": wqT, "wkT": wkT, "wvT": wvT, "woT": woT,
        "bq": bq, "bk": np.ascontiguousarray(bkv[:C]),
        "bvb": bvb, "bob": bob,
    }
    return [
        dict(
            shared,
            xT=np.ascontiguousarray(x[b].T.astype(bf)),
            encT=np.ascontiguousarray(enc[b].T.astype(bf)),
        )
        for b in range(x.shape[0])
    ]


def kernel(x, encoder_output, Wq, bq, Wkv, bkv, Wo, bo):
    from concourse.bass_utils import run_bass_kernel_spmd

    nc = _get_nc()
    in_maps = _prep_in_maps(x, encoder_output, Wq, bq, Wkv, bkv, Wo, bo)
    res = run_bass_kernel_spmd(nc, in_maps, list(range(len(in_maps)))).results
    return np.stack([res[b]["out"] for b in range(len(res))]).astype(np.float32)



# revision 6
# speedup vs baseline: 1.2669x; 1.2669x over previous
"""Cross-attention kernel for Trainium2, 8 NeuronCores, data-parallel over
batch (B=8 == n_cores, one batch element per core, no collectives).

v2 design (from the 266us v1):
  - S^T computed per head-PAIR with two CONCURRENT row-tiled matmuls
    (tile_position (0,0) / (64,0), 64x128 mode): even head -> cols 0:512,
    odd head -> cols 512:1024 of ONE [128,1024] psum tile; ONE exp per
    pair-chunk tile.  (Row-tile concurrency measured at 20ns issue spacing
    on this HW for v1's tail pairs.)
  - Uniform i-tail: KT padded to 640 cols (zeros), V3 tail rows 64:128
    zeroed (incl. ones col) so the padded i rows contribute exp(0)*0 = 0
    to both y and Z.  No special tail mode, no V3 row duplication.
  - 1/Z broadcast as two concurrent rank-1 matmuls per t-half
    ((32,64)-mode tiles at positions (0,0) and (32,64)) + ONE pair-wide
    DVE multiply of YT[oc] [128,1024].
  - bo folded into the out-proj drain (DVE tensor_add with a broadcast
    bias tile); no rank-1 bias matmuls.
  - Inputs loaded with FEW large DMAs (HWDGE issue costs ~1.2us each):
    sync ring: wk cols 0:128, wk cols 128:, bvb, wv, bobb, wo(late);
    scalar ring: encT, wq cols 0:128, xT half 0, xT half 1, wq cols 128:.
    Tiny bias gathers ride the gpsimd SWDGE ring after its memsets.
  - Short warmup (HAM un-throttle) that gates only on two small memsets.
"""

import numpy as np

T = 1024
C = 1024
I = 576
IP = 640                # padded i (5 full 128-chunks)
H = 16
D = 64
NCC = C // 128          # 8 contraction chunks
VW = 66                 # per-head column block in V tile: 64 v cols + ones col + pad
SCALE = 1.0 / np.sqrt(D)

_CACHE = {}


def _build_nc():
    import concourse.bass as bass
    import concourse.bacc as bacc
    import concourse.mybir as mybir
    import concourse.tile as tile
    from contextlib import ExitStack

    f32 = mybir.dt.float32
    bf16 = mybir.dt.bfloat16

    nc = bacc.Bacc()

    xT_d = nc.dram_tensor("xT", [C, T], bf16, kind="ExternalInput")
    encT_d = nc.dram_tensor("encT", [C, I], bf16, kind="ExternalInput")
    wqT_d = nc.dram_tensor("wqT", [C, C], bf16, kind="ExternalInput")
    wkT_d = nc.dram_tensor("wkT", [C, C], bf16, kind="ExternalInput")
    wvT_d = nc.dram_tensor("wvT", [C, C], bf16, kind="ExternalInput")
    woT_d = nc.dram_tensor("woT", [C, C], bf16, kind="ExternalInput")
    bq_d = nc.dram_tensor("bq", [C], f32, kind="ExternalInput")
    bk_d = nc.dram_tensor("bk", [C], f32, kind="ExternalInput")
    bvb_d = nc.dram_tensor("bvb", [128, C], f32, kind="ExternalInput")
    bobb_d = nc.dram_tensor("bobb", [128, C], bf16, kind="ExternalInput")
    out_d = nc.dram_tensor("out", [T, C], bf16, kind="ExternalOutput")

    with ExitStack() as ctx:
        tc = ctx.enter_context(tile.TileContext(nc))

        resid = ctx.enter_context(tc.tile_pool(name="resid", bufs=1))
        misc = ctx.enter_context(tc.tile_pool(name="misc", bufs=1))
        ps_s = ctx.enter_context(tc.tile_pool(name="ps_s", bufs=2, space="PSUM"))
        ps_av = ctx.enter_context(tc.tile_pool(name="ps_av", bufs=1, space="PSUM"))
        ps_pj = ctx.enter_context(tc.tile_pool(name="ps_pj", bufs=1, space="PSUM"))
        exps = ctx.enter_context(tc.tile_pool(name="exps", bufs=22))
        osb = ctx.enter_context(tc.tile_pool(name="osb", bufs=2))
        wvp = ctx.enter_context(tc.tile_pool(name="wvp", bufs=1))
        zrp = ctx.enter_context(tc.tile_pool(name="zrp", bufs=2))

        # ---- resident tensors ----
        zmask = misc.tile([33, 64], bf16)     # rows 0 / 32 used as rank-1 ones
        warm = misc.tile([1, 512], bf16)
        bq_t = misc.tile([128, NCC], f32)
        bk_t = misc.tile([128, NCC], f32)
        bvb = misc.tile([128, C], f32)
        bobc = misc.tile([128, C], bf16)

        wk = resid.tile([128, NCC, C], bf16, name="wk")
        wq = resid.tile([128, NCC, C], bf16, name="wq")
        xT = resid.tile([128, NCC, T], bf16, name="xT")
        encT = resid.tile([128, NCC, IP], bf16, name="encT")
        QT = [resid.tile([128, T], bf16, tag=f"QT{i}", name=f"QT{i}") for i in range(NCC)]
        KT = [resid.tile([128, IP], bf16, tag=f"KT{i}", name=f"KT{i}") for i in range(NCC)]
        V3 = [resid.tile([128, H, VW], bf16, tag=f"V{i}", name=f"V{i}") for i in range(5)]
        YT = [resid.tile([128, T], bf16, tag=f"YT{i}", name=f"YT{i}") for i in range(NCC)]
        # wo reuses wv's SBUF (same tag, bufs=1): its DMA waits until the
        # last V-proj matmul has consumed wv.
        wv = wvp.tile([128, NCC, C], bf16, tag="wv", name="wv")

        # ---- gpsimd: memsets first (two small ones gate the PE warmup),
        # then the tiny bias gathers on the SWDGE ring ----
        nc.gpsimd.memset(zmask, 1.0)
        nc.gpsimd.memset(warm, 1.0)
        nc.gpsimd.dma_start(out=bq_t, in_=bq_d[:].rearrange("(oc p) -> p oc", p=128))
        nc.gpsimd.dma_start(out=bk_t, in_=bk_d[:].rearrange("(oc p) -> p oc", p=128))
        for oc in range(NCC):
            nc.gpsimd.memset(KT[oc][:, I:IP], 0.0)
        for ii in range(4):
            nc.gpsimd.memset(V3[ii][:, :, 64:65], 1.0)
        nc.gpsimd.memset(V3[4], 0.0)
        nc.gpsimd.memset(V3[4][0:64, :, 64:65], 1.0)

        # ---- input DMAs: few big transfers, need-ordered ----
        wkT_r = wkT_d[:, :].rearrange("(cc p) c -> p cc c", p=128)
        wqT_r = wqT_d[:, :].rearrange("(cc p) c -> p cc c", p=128)
        wvT_r = wvT_d[:, :].rearrange("(cc p) c -> p cc c", p=128)
        woT_r = woT_d[:, :].rearrange("(cc p) c -> p cc c", p=128)
        xT_r = xT_d[:, :].rearrange("(cc p) t -> p cc t", p=128)
        encT_r = encT_d[:, :].rearrange("(cc p) i -> p cc i", p=128)

        nc.sync.dma_start(out=wk[:, :, 0:128], in_=wkT_r[:, :, 0:128])
        nc.sync.dma_start(out=wk[:, :, 128:256], in_=wkT_r[:, :, 128:256])
        nc.sync.dma_start(out=bvb, in_=bvb_d[:, :])
        nc.sync.dma_start(out=wv, in_=wvT_r)
        nc.sync.dma_start(out=wk[:, :, 256:C], in_=wkT_r[:, :, 256:C])
        nc.sync.dma_start(out=bobc, in_=bobb_d[:, :])

        nc.scalar.dma_start(out=encT[:, :, 0:I], in_=encT_r)
        nc.scalar.dma_start(out=wq[:, :, 0:128], in_=wqT_r[:, :, 0:128])
        nc.scalar.dma_start(out=xT[:, :, 0:512], in_=xT_r[:, :, 0:512])
        nc.scalar.dma_start(out=xT[:, :, 512:T], in_=xT_r[:, :, 512:T])
        nc.scalar.dma_start(out=wq[:, :, 128:256], in_=wqT_r[:, :, 128:256])
        nc.scalar.dma_start(out=wq[:, :, 256:C], in_=wqT_r[:, :, 256:C])

        # ---- PE p-state warmup while the first input DMAs stream ----
        for w in range(10):
            pw = ps_pj.tile([128, 1024], f32, tag="pj")
            nc.tensor.matmul(pw[:64, 0:512], zmask[0:1, 0:64], warm,
                             start=True, stop=True)

        # ---- building blocks ----
        def k_proj(oc):
            pk = ps_pj.tile([128, 1024], f32, tag="pj")
            for cc in range(NCC):
                for ih in range(2):
                    nc.tensor.matmul(
                        pk[:, ih * 512 : ih * 512 + 288],
                        wk[:, cc, oc * 128 : (oc + 1) * 128],
                        encT[:, cc, ih * 288 : (ih + 1) * 288],
                        start=(cc == 0),
                        stop=(cc == NCC - 1),
                    )
            for ih in range(2):
                nc.vector.tensor_scalar_add(
                    KT[oc][:, ih * 288 : (ih + 1) * 288],
                    pk[:, ih * 512 : ih * 512 + 288],
                    bk_t[:, oc : oc + 1],
                )

        def q_half(oc, tch):
            tsl = slice(tch * 512, (tch + 1) * 512)
            pq = ps_pj.tile([128, 1024], f32, tag="pj")
            for cc in range(NCC):
                nc.tensor.matmul(
                    pq[:, 0:512],
                    wq[:, cc, oc * 128 : (oc + 1) * 128],
                    xT[:, cc, tsl],
                    start=(cc == 0),
                    stop=(cc == NCC - 1),
                )
            nc.vector.tensor_scalar_add(QT[oc][:, tsl], pq[:, 0:512], bq_t[:, oc : oc + 1])

        def v_group(och, ii):
            """V3[ii][:, 8*och:8*och+8, 0:64] = (enc @ WvT + bv) block."""
            pi = 128 if ii < 4 else 64
            osl = slice(och * 512, (och + 1) * 512)
            pv = ps_pj.tile([128, 1024], f32, tag="pj")
            for cc in range(NCC):
                nc.tensor.matmul(
                    pv[:pi, 0:512],
                    encT[:, cc, ii * 128 : ii * 128 + pi],
                    wv[:, cc, osl],
                    start=(cc == 0),
                    stop=(cc == NCC - 1),
                )
            nc.vector.tensor_add(
                V3[ii][:pi, och * 8 : och * 8 + 8, 0:64],
                pv[:pi, 0:512].rearrange("p (h d) -> p h d", d=64),
                bvb[:pi, osl].rearrange("p (h d) -> p h d", d=64),
            )

        def s_slot(oc, ii, tch):
            """Both heads of pair oc, one i-chunk, one t-half: 2 concurrent
            row-tiled matmuls into one psum tile + ONE exp."""
            sp = ps_s.tile([128, 1024], f32, tag="s")
            for hb, cs in ((0, 0), (64, 512)):
                nc.tensor.matmul(
                    sp[:, cs : cs + 512],
                    KT[oc][hb : hb + 64, ii * 128 : (ii + 1) * 128],
                    QT[oc][hb : hb + 64, tch * 512 : (tch + 1) * 512],
                    start=True,
                    stop=True,
                )
            e = exps.tile([128, 1024], bf16, tag="exps")
            nc.scalar.activation(
                e, sp, mybir.ActivationFunctionType.Exp, scale=float(SCALE)
            )
            return e

        def av_half(h, E, tch, py):
            cs = (h % 2) * 512
            tsl = slice(tch * 512, (tch + 1) * 512)
            for ii in range(5):
                nc.tensor.matmul(
                    py[:65, tsl],
                    V3[ii][:, h, 0:65],
                    E[ii][tch][:, cs : cs + 512],
                    start=(ii == 0),
                    stop=(ii == 4),
                )

        zpair = {}

        def av_drain(h, py, zt2, last=False):
            oc, hb = h // 2, (h % 2) * 64
            zb = (h % 2) * 32
            eng = nc.scalar if last else nc.vector
            if last:
                eng.copy(YT[oc][hb : hb + 64, :], py[:64, :])
            else:
                eng.tensor_copy(YT[oc][hb : hb + 64, :], py[:64, :])
            nc.vector.tensor_copy(zt2[zb : zb + 1, :], py[64:65, :])

        def norm_pair(oc):
            """pb[0:64] = 1/Z_even bcast, pb[64:128] = 1/Z_odd bcast via two
            concurrent (32,64)-tile rank-1 matmuls per t-half, then ONE
            pair-wide DVE multiply of YT[oc]."""
            zt2 = zpair[oc]
            nc.vector.reciprocal_approx_fast(out=zt2, in_=zt2)
            zr2 = zrp.tile([33, T], bf16, tag="zr", name=f"zr{oc}")
            nc.vector.tensor_copy(zr2[0:1, :], zt2[0:1, :])
            nc.vector.tensor_copy(zr2[32:33, :], zt2[32:33, :])
            pb = ps_av.tile([128, 1024], f32, tag="av")
            for tch in range(2):
                tsl = slice(tch * 512, (tch + 1) * 512)
                nc.tensor.matmul(
                    pb[0:64, tsl], zmask[0:1, :], zr2[0:1, tsl],
                    start=True, stop=True,
                )
                nc.tensor.matmul(
                    pb[64:128, tsl], zmask[32:33, :], zr2[32:33, tsl],
                    start=True, stop=True,
                )
            nc.vector.tensor_mul(YT[oc], YT[oc], pb)

        # ---- pre-loop: projections for pairs 0/1, S for pair 0, V och=0 ----
        E = {p: [[None, None] for _ in range(5)] for p in range(8)}

        def s_emit(p, ii, tch):
            E[p][ii][tch] = s_slot(p, ii, tch)

        k_proj(0)
        k_proj(1)
        q_half(0, 0)
        for ii in range(5):
            s_emit(0, ii, 0)
        q_half(0, 1)
        for ii in range(5):
            s_emit(0, ii, 1)
            v_group(0, ii)
        q_half(1, 0)
        q_half(1, 1)
        for ii in range(3):
            v_group(1, ii)

        # ---- main rounds: av(pair oc) + S(pair oc+1) + proj(pair oc+2) ----
        for oc in range(8):
            p1, p2 = oc + 1, oc + 2
            live1, live2 = p1 < 8, p2 < 8
            last = oc == 7
            if live1:
                s_emit(p1, 0, 0)
                s_emit(p1, 1, 0)
            if live2:
                k_proj(p2)
            if live1:
                s_emit(p1, 2, 0)
            py0 = ps_av.tile([128, 1024], f32, tag="av")
            av_half(2 * oc, E[oc], 0, py0)
            if live1:
                s_emit(p1, 3, 0)
            if oc == 0:
                v_group(1, 3)
            av_half(2 * oc, E[oc], 1, py0)
            zpair[oc] = zrp.tile([33, T], f32, tag="zt", name=f"zt{oc}")
            av_drain(2 * oc, py0, zpair[oc], last=last)
            if live1:
                s_emit(p1, 4, 0)
                s_emit(p1, 0, 1)
            if live2:
                q_half(p2, 0)
                q_half(p2, 1)
            if oc == 0:
                v_group(1, 4)
            if live1:
                s_emit(p1, 1, 1)
            py1 = ps_av.tile([128, 1024], f32, tag="av")
            av_half(2 * oc + 1, E[oc], 0, py1)
            if live1:
                s_emit(p1, 2, 1)
                s_emit(p1, 3, 1)
            av_half(2 * oc + 1, E[oc], 1, py1)
            av_drain(2 * oc + 1, py1, zpair[oc], last=last)
            if live1:
                s_emit(p1, 4, 1)
            norm_pair(oc)

        # ---- output projection ----
        wo = wvp.tile([128, NCC, C], bf16, tag="wv", name="wo")
        nc.sync.dma_start(out=wo, in_=woT_r)
        for tt in range(8):
            po = ps_pj.tile([128, 1024], f32, tag="pj")
            for cc in range(NCC):
                for och in range(2):
                    osl = slice(och * 512, (och + 1) * 512)
                    nc.tensor.matmul(
                        po[:, osl],
                        YT[cc][:, tt * 128 : (tt + 1) * 128],
                        wo[:, cc, osl],
                        start=(cc == 0),
                        stop=(cc == NCC - 1),
                    )
            ot = osb.tile([128, C], bf16, tag="osb")
            nc.vector.tensor_add(ot, po, bobc)
            eng = nc.sync if tt % 2 == 0 else nc.scalar
            eng.dma_start(out=out_d[tt * 128 : (tt + 1) * 128], in_=ot)

    nc.compile()
    return nc


def _get_nc():
    if "nc" not in _CACHE:
        _CACHE["nc"] = _build_nc()
    return _CACHE["nc"]


def _prep_in_maps(x, encoder_output, Wq, bq, Wkv, bkv, Wo, bo):
    import ml_dtypes
    f = np.float32
    bf = ml_dtypes.bfloat16
    x = np.asarray(x, f)
    enc = np.asarray(encoder_output, f)
    wqT = np.ascontiguousarray(np.asarray(Wq, f).T.astype(bf))
    wkv = np.asarray(Wkv, f)
    wkT = np.ascontiguousarray(wkv[:C].T.astype(bf))
    wvT = np.ascontiguousarray(wkv[C:].T.astype(bf))
    woT = np.ascontiguousarray(np.asarray(Wo, f).T.astype(bf))
    bq = np.asarray(bq, f)
    bkv = np.asarray(bkv, f)
    bo = np.asarray(bo, f)
    bvb = np.ascontiguousarray(np.broadcast_to(bkv[C:], (128, C)).astype(f))
    bobb = np.ascontiguousarray(np.broadcast_to(bo, (128, C)).astype(bf))
    shared = {
        "wqT": wqT, "wkT": wkT, "wvT": wvT, "woT": woT,
        "bq": bq, "bk": np.ascontiguousarray(bkv[:C]),
        "bvb": bvb, "bobb": bobb,
    }
    return [
        dict(
            shared,
            xT=np.ascontiguousarray(x[b].T.astype(bf)),
            encT=np.ascontiguousarray(enc[b].T.astype(bf)),
        )
        for b in range(x.shape[0])
    ]


def kernel(x, encoder_output, Wq, bq, Wkv, bkv, Wo, bo):
    from concourse.bass_utils import run_bass_kernel_spmd

    nc = _get_nc()
    in_maps = _prep_in_maps(x, encoder_output, Wq, bq, Wkv, bkv, Wo, bo)
    res = run_bass_kernel_spmd(nc, in_maps, list(range(len(in_maps)))).results
    return np.stack([res[b]["out"] for b in range(len(res))]).astype(np.float32)


# revision 17
# speedup vs baseline: 1.3430x; 1.0601x over previous
"""Cross-attention kernel for Trainium2, 8 NeuronCores, data-parallel over
batch (B=8 == n_cores, one batch element per core, no collectives).

v2 design (from the 266us v1):
  - S^T computed per head-PAIR with two CONCURRENT row-tiled matmuls
    (tile_position (0,0) / (64,0), 64x128 mode): even head -> cols 0:512,
    odd head -> cols 512:1024 of ONE [128,1024] psum tile; ONE exp per
    pair-chunk tile.  (Row-tile concurrency measured at 20ns issue spacing
    on this HW for v1's tail pairs.)
  - Uniform i-tail: KT padded to 640 cols (zeros), V3 tail rows 64:128
    zeroed (incl. ones col) so the padded i rows contribute exp(0)*0 = 0
    to both y and Z.  No special tail mode, no V3 row duplication.
  - 1/Z broadcast as two concurrent rank-1 matmuls per t-half
    ((32,64)-mode tiles at positions (0,0) and (32,64)) + ONE pair-wide
    DVE multiply of YT[oc] [128,1024].
  - bo folded into the out-proj drain (DVE tensor_add with a broadcast
    bias tile); no rank-1 bias matmuls.
  - Inputs loaded with FEW large DMAs (HWDGE issue costs ~1.2us each):
    sync ring: wk cols 0:128, wk cols 128:, bvb, wv, bobb, wo(late);
    scalar ring: encT, wq cols 0:128, xT half 0, xT half 1, wq cols 128:.
    Tiny bias gathers ride the gpsimd SWDGE ring after its memsets.
  - Short warmup (HAM un-throttle) that gates only on two small memsets.
"""

import numpy as np

T = 1024
C = 1024
I = 576
IP = 640                # padded i (5 full 128-chunks)
H = 16
D = 64
NCC = C // 128          # 8 contraction chunks
VW = 66                 # per-head column block in V tile: 64 v cols + ones col + pad
SCALE = 1.0 / np.sqrt(D)

_CACHE = {}


def _build_nc():
    import concourse.bass as bass
    import concourse.bacc as bacc
    import concourse.mybir as mybir
    import concourse.tile as tile
    from contextlib import ExitStack

    f32 = mybir.dt.float32
    bf16 = mybir.dt.bfloat16

    nc = bacc.Bacc()

    xT_d = nc.dram_tensor("xT", [C, T], bf16, kind="ExternalInput")
    encT_d = nc.dram_tensor("encT", [C, I], bf16, kind="ExternalInput")
    wqT_d = nc.dram_tensor("wqT", [C, C], bf16, kind="ExternalInput")
    wkT_d = nc.dram_tensor("wkT", [C, C], bf16, kind="ExternalInput")
    wvT_d = nc.dram_tensor("wvT", [C, C], bf16, kind="ExternalInput")
    woT_d = nc.dram_tensor("woT", [C, C], bf16, kind="ExternalInput")
    bq_d = nc.dram_tensor("bq", [C], f32, kind="ExternalInput")
    bk_d = nc.dram_tensor("bk", [C], f32, kind="ExternalInput")
    bvb_d = nc.dram_tensor("bvb", [128, C], f32, kind="ExternalInput")
    bobb_d = nc.dram_tensor("bobb", [128, C], bf16, kind="ExternalInput")
    out_d = nc.dram_tensor("out", [T, C], bf16, kind="ExternalOutput")

    with ExitStack() as ctx:
        tc = ctx.enter_context(tile.TileContext(nc))

        resid = ctx.enter_context(tc.tile_pool(name="resid", bufs=1))
        misc = ctx.enter_context(tc.tile_pool(name="misc", bufs=1))
        ps_s = ctx.enter_context(tc.tile_pool(name="ps_s", bufs=2, space="PSUM"))
        ps_av = ctx.enter_context(tc.tile_pool(name="ps_av", bufs=1, space="PSUM"))
        ps_pj = ctx.enter_context(tc.tile_pool(name="ps_pj", bufs=1, space="PSUM"))
        exps = ctx.enter_context(tc.tile_pool(name="exps", bufs=27))
        osb = ctx.enter_context(tc.tile_pool(name="osb", bufs=2))
        wvp = ctx.enter_context(tc.tile_pool(name="wvp", bufs=1))
        zrp = ctx.enter_context(tc.tile_pool(name="zrp", bufs=2))

        # ---- resident tensors ----
        zmask = misc.tile([33, 64], bf16)     # rows 0 / 32 used as rank-1 ones
        warm = misc.tile([1, 512], bf16)
        bq_t = misc.tile([128, NCC], f32)
        bk_t = misc.tile([128, NCC], f32)
        bvb = misc.tile([128, C], f32)
        bobc = misc.tile([128, C], bf16)

        wk = resid.tile([128, NCC, C], bf16, name="wk")
        wq = resid.tile([128, NCC, C], bf16, name="wq")
        xT = resid.tile([128, NCC, T], bf16, name="xT")
        encT = resid.tile([128, NCC, IP], bf16, name="encT")
        QT = [resid.tile([128, T], bf16, tag=f"QT{i}", name=f"QT{i}") for i in range(NCC)]
        KT = [resid.tile([128, IP], bf16, tag=f"KT{i}", name=f"KT{i}") for i in range(NCC)]
        V3 = [resid.tile([128, H, VW], bf16, tag=f"V{i}", name=f"V{i}") for i in range(5)]
        YT = [resid.tile([128, T], bf16, tag=f"YT{i}", name=f"YT{i}") for i in range(NCC)]
        # wo reuses wv's SBUF (same tag, bufs=1): its DMA waits until the
        # last V-proj matmul has consumed wv.
        wv = wvp.tile([128, NCC, C], bf16, tag="wv", name="wv")

        # ---- gpsimd: memsets first (two small ones gate the PE warmup),
        # then the tiny bias gathers on the SWDGE ring ----
        nc.gpsimd.memset(zmask, 1.0)
        nc.gpsimd.memset(warm, 1.0)
        nc.gpsimd.dma_start(out=bq_t, in_=bq_d[:].rearrange("(oc p) -> p oc", p=128))
        nc.gpsimd.dma_start(out=bk_t, in_=bk_d[:].rearrange("(oc p) -> p oc", p=128))
        for oc in range(NCC):
            nc.gpsimd.memset(KT[oc][:, I:IP], 0.0)
        for ii in range(4):
            nc.gpsimd.memset(V3[ii][:, :, 64:65], 1.0)
        nc.gpsimd.memset(V3[4], 0.0)
        nc.gpsimd.memset(V3[4][0:64, :, 64:65], 1.0)

        # ---- input DMAs: few big transfers, need-ordered ----
        wkT_r = wkT_d[:, :].rearrange("(cc p) c -> p cc c", p=128)
        wqT_r = wqT_d[:, :].rearrange("(cc p) c -> p cc c", p=128)
        wvT_r = wvT_d[:, :].rearrange("(cc p) c -> p cc c", p=128)
        woT_r = woT_d[:, :].rearrange("(cc p) c -> p cc c", p=128)
        xT_r = xT_d[:, :].rearrange("(cc p) t -> p cc t", p=128)
        encT_r = encT_d[:, :].rearrange("(cc p) i -> p cc i", p=128)

        nc.sync.dma_start(out=wk[:, :, 0:128], in_=wkT_r[:, :, 0:128])
        nc.sync.dma_start(out=wk[:, :, 128:256], in_=wkT_r[:, :, 128:256])
        nc.sync.dma_start(out=bvb, in_=bvb_d[:, :])
        nc.sync.dma_start(out=wv, in_=wvT_r)
        nc.sync.dma_start(out=wk[:, :, 256:C], in_=wkT_r[:, :, 256:C])
        nc.sync.dma_start(out=bobc, in_=bobb_d[:, :])

        nc.scalar.dma_start(out=encT[:, :, 0:I], in_=encT_r)
        nc.scalar.dma_start(out=wq[:, :, 0:128], in_=wqT_r[:, :, 0:128])
        nc.scalar.dma_start(out=xT[:, :, 0:512], in_=xT_r[:, :, 0:512])
        nc.scalar.dma_start(out=xT[:, :, 512:T], in_=xT_r[:, :, 512:T])
        nc.scalar.dma_start(out=wq[:, :, 128:256], in_=wqT_r[:, :, 128:256])
        nc.scalar.dma_start(out=wq[:, :, 256:C], in_=wqT_r[:, :, 256:C])

        # ---- PE p-state warmup while the first input DMAs stream ----
        for w in range(10):
            pw = ps_pj.tile([128, 1024], f32, tag="pj")
            nc.tensor.matmul(pw[:64, 0:512], zmask[0:1, 0:64], warm,
                             start=True, stop=True)

        # ---- building blocks ----
        def k_proj(oc):
            pk = ps_pj.tile([128, 1024], f32, tag="pj")
            for cc in range(NCC):
                for ih in range(2):
                    nc.tensor.matmul(
                        pk[:, ih * 512 : ih * 512 + 288],
                        wk[:, cc, oc * 128 : (oc + 1) * 128],
                        encT[:, cc, ih * 288 : (ih + 1) * 288],
                        start=(cc == 0),
                        stop=(cc == NCC - 1),
                    )
            for ih in range(2):
                nc.vector.tensor_scalar_add(
                    KT[oc][:, ih * 288 : (ih + 1) * 288],
                    pk[:, ih * 512 : ih * 512 + 288],
                    bk_t[:, oc : oc + 1],
                )

        def q_half(oc, tch):
            tsl = slice(tch * 512, (tch + 1) * 512)
            pq = ps_pj.tile([128, 1024], f32, tag="pj")
            for cc in range(NCC):
                nc.tensor.matmul(
                    pq[:, 0:512],
                    wq[:, cc, oc * 128 : (oc + 1) * 128],
                    xT[:, cc, tsl],
                    start=(cc == 0),
                    stop=(cc == NCC - 1),
                )
            nc.vector.tensor_scalar_add(QT[oc][:, tsl], pq[:, 0:512], bq_t[:, oc : oc + 1])

        def v_group(och, ii):
            """V3[ii][:, 8*och:8*och+8, 0:64] = (enc @ WvT + bv) block."""
            pi = 128 if ii < 4 else 64
            osl = slice(och * 512, (och + 1) * 512)
            pv = ps_pj.tile([128, 1024], f32, tag="pj")
            for cc in range(NCC):
                nc.tensor.matmul(
                    pv[:pi, 0:512],
                    encT[:, cc, ii * 128 : ii * 128 + pi],
                    wv[:, cc, osl],
                    start=(cc == 0),
                    stop=(cc == NCC - 1),
                )
            nc.vector.tensor_add(
                V3[ii][:pi, och * 8 : och * 8 + 8, 0:64],
                pv[:pi, 0:512].rearrange("p (h d) -> p h d", d=64),
                bvb[:pi, osl].rearrange("p (h d) -> p h d", d=64),
            )

        def s_slot(oc, ii, tch):
            """Both heads of pair oc, one i-chunk, one t-half: 2 concurrent
            row-tiled matmuls into one psum tile + ONE exp."""
            sp = ps_s.tile([128, 1024], f32, tag="s")
            for hb, cs in ((0, 0), (64, 512)):
                nc.tensor.matmul(
                    sp[:, cs : cs + 512],
                    KT[oc][hb : hb + 64, ii * 128 : (ii + 1) * 128],
                    QT[oc][hb : hb + 64, tch * 512 : (tch + 1) * 512],
                    start=True,
                    stop=True,
                )
            e = exps.tile([128, 1024], bf16, tag="exps")
            nc.scalar.activation(
                e, sp, mybir.ActivationFunctionType.Exp, scale=float(SCALE)
            )
            return e

        def av_half(h, E, tch, py):
            cs = (h % 2) * 512
            tsl = slice(tch * 512, (tch + 1) * 512)
            for ii in range(5):
                nc.tensor.matmul(
                    py[:65, tsl],
                    V3[ii][:, h, 0:65],
                    E[ii][tch][:, cs : cs + 512],
                    start=(ii == 0),
                    stop=(ii == 4),
                )

        zpair = {}

        def av_drain(h, py, zt2, last=False):
            oc, hb = h // 2, (h % 2) * 64
            zb = (h % 2) * 32
            eng = nc.scalar if last else nc.vector
            if last:
                eng.copy(YT[oc][hb : hb + 64, :], py[:64, :])
            else:
                eng.tensor_copy(YT[oc][hb : hb + 64, :], py[:64, :])
            nc.vector.tensor_copy(zt2[zb : zb + 1, :], py[64:65, :])

        zrr = {}

        def norm_stage(oc, split):
            """1/Z on DVE: recip + bf16 cast.  split=True does it per half
            (right after each head's drain) to shorten the end-of-kernel
            dependency chain."""
            zt2 = zpair[oc]
            if oc not in zrr:
                zrr[oc] = zrp.tile([33, T], bf16, tag="zr", name=f"zr{oc}")
            zr2 = zrr[oc]
            if split is None:
                nc.vector.reciprocal_approx_fast(out=zt2, in_=zt2)
                nc.vector.tensor_copy(zr2[0:1, :], zt2[0:1, :])
                nc.vector.tensor_copy(zr2[32:33, :], zt2[32:33, :])
            else:
                zb = split * 32
                nc.vector.reciprocal_approx_fast(
                    out=zt2[zb : zb + 1, :], in_=zt2[zb : zb + 1, :]
                )
                nc.vector.tensor_copy(zr2[zb : zb + 1, :], zt2[zb : zb + 1, :])

        def norm_mm(oc):
            """pb[0:64] = 1/Z_even bcast, pb[64:128] = 1/Z_odd bcast via two
            concurrent (32,64)-tile rank-1 matmuls per t-half, then ONE
            pair-wide DVE multiply of YT[oc]."""
            zr2 = zrr[oc]
            pb = ps_av.tile([128, 1024], f32, tag="av")
            for tch in range(2):
                tsl = slice(tch * 512, (tch + 1) * 512)
                nc.tensor.matmul(
                    pb[0:64, tsl], zmask[0:1, :], zr2[0:1, tsl],
                    start=True, stop=True,
                )
                nc.tensor.matmul(
                    pb[64:128, tsl], zmask[32:33, :], zr2[32:33, tsl],
                    start=True, stop=True,
                )
            nc.vector.tensor_mul(YT[oc], YT[oc], pb)

        # ---- pre-loop: projections for pairs 0/1, S for pair 0, V och=0 ----
        E = {p: [[None, None] for _ in range(5)] for p in range(8)}

        def s_emit(p, ii, tch):
            E[p][ii][tch] = s_slot(p, ii, tch)

        k_proj(0)
        k_proj(1)
        q_half(0, 0)
        for ii in range(5):
            s_emit(0, ii, 0)
        q_half(0, 1)
        for ii in range(5):
            s_emit(0, ii, 1)
            v_group(0, ii)
        q_half(1, 0)
        q_half(1, 1)
        for ii in range(3):
            v_group(1, ii)

        # ---- main rounds ----
        # Round oc: av(pair oc); S(pair oc+1) t-half 1 + S(pair oc+2) t-half
        # 0 (its K/Q proj drains mid-round); projections for pair oc+2.
        # The half-early S emission keeps the Act exp queue fed so the last
        # pairs' AVs aren't exp-gated once projection work runs out.
        for oc in range(8):
            p1, p2 = oc + 1, oc + 2
            live1, live2 = p1 < 8, p2 < 8
            last = oc == 7
            if live1:
                s_emit(p1, 0, 0)
                s_emit(p1, 1, 0)
            if live2:
                k_proj(p2)
            if live1:
                s_emit(p1, 2, 0)
            py0 = ps_av.tile([128, 1024], f32, tag="av")
            av_half(2 * oc, E[oc], 0, py0)
            if live1:
                s_emit(p1, 3, 0)
            if oc == 0:
                v_group(1, 3)
            av_half(2 * oc, E[oc], 1, py0)
            zpair[oc] = zrp.tile([33, T], f32, tag="zt", name=f"zt{oc}")
            av_drain(2 * oc, py0, zpair[oc], last=last)
            if live1:
                s_emit(p1, 4, 0)
                s_emit(p1, 0, 1)
            if live2:
                q_half(p2, 0)
                q_half(p2, 1)
            if oc == 0:
                v_group(1, 4)
            if live1:
                s_emit(p1, 1, 1)
            py1 = ps_av.tile([128, 1024], f32, tag="av")
            av_half(2 * oc + 1, E[oc], 0, py1)
            if live1:
                s_emit(p1, 2, 1)
                s_emit(p1, 3, 1)
            av_half(2 * oc + 1, E[oc], 1, py1)
            av_drain(2 * oc + 1, py1, zpair[oc], last=last)
            if live1:
                s_emit(p1, 4, 1)
            norm_stage(oc, None)
            norm_mm(oc)

        # ---- output projection ----
        # po rotates in the (now idle) 2-buf S psum pool so tt+1's matmuls
        # overlap tt's DVE drain.  Pair 7's norm matmuls are woven after
        # tt=0's cc 0..6 so its DVE recip/mul chain hides under real work;
        # tt 0/1 emit cc=7 last so YT[7]'s multiply has a ~3us window.
        wo = wvp.tile([128, NCC, C], bf16, tag="wv", name="wo")
        nc.sync.dma_start(out=wo, in_=woT_r)
        for tt in range(8):
            po = ps_s.tile([128, 1024], f32, tag="s")
            for cc in range(NCC):
                for och in range(2):
                    osl = slice(och * 512, (och + 1) * 512)
                    nc.tensor.matmul(
                        po[:, osl],
                        YT[cc][:, tt * 128 : (tt + 1) * 128],
                        wo[:, cc, osl],
                        start=(cc == 0),
                        stop=(cc == NCC - 1),
                    )
            ot = osb.tile([128, C], bf16, tag="osb")
            nc.vector.tensor_add(ot, po, bobc)
            eng = nc.sync if tt % 2 == 0 else nc.scalar
            eng.dma_start(out=out_d[tt * 128 : (tt + 1) * 128], in_=ot)

    nc.compile()
    return nc


def _get_nc():
    if "nc" not in _CACHE:
        _CACHE["nc"] = _build_nc()
    return _CACHE["nc"]


def _prep_in_maps(x, encoder_output, Wq, bq, Wkv, bkv, Wo, bo):
    import ml_dtypes
    f = np.float32
    bf = ml_dtypes.bfloat16
    x = np.asarray(x, f)
    enc = np.asarray(encoder_output, f)
    wqT = np.ascontiguousarray(np.asarray(Wq, f).T.astype(bf))
    wkv = np.asarray(Wkv, f)
    wkT = np.ascontiguousarray(wkv[:C].T.astype(bf))
    wvT = np.ascontiguousarray(wkv[C:].T.astype(bf))
    woT = np.ascontiguousarray(np.asarray(Wo, f).T.astype(bf))
    bq = np.asarray(bq, f)
    bkv = np.asarray(bkv, f)
    bo = np.asarray(bo, f)
    bvb = np.ascontiguousarray(np.broadcast_to(bkv[C:], (128, C)).astype(f))
    bobb = np.ascontiguousarray(np.broadcast_to(bo, (128, C)).astype(bf))
    shared = {
        "wqT": wqT, "wkT": wkT, "wvT": wvT, "woT": woT,
        "bq": bq, "bk": np.ascontiguousarray(bkv[:C]),
        "bvb": bvb, "bobb": bobb,
    }
    return [
        dict(
            shared,
            xT=np.ascontiguousarray(x[b].T.astype(bf)),
            encT=np.ascontiguousarray(enc[b].T.astype(bf)),
        )
        for b in range(x.shape[0])
    ]


def kernel(x, encoder_output, Wq, bq, Wkv, bkv, Wo, bo):
    from concourse.bass_utils import run_bass_kernel_spmd

    nc = _get_nc()
    in_maps = _prep_in_maps(x, encoder_output, Wq, bq, Wkv, bkv, Wo, bo)
    res = run_bass_kernel_spmd(nc, in_maps, list(range(len(in_maps)))).results
    return np.stack([res[b]["out"] for b in range(len(res))]).astype(np.float32)
